# revision 2
# baseline (speedup 1.0000x reference)
"""BoxQueryAndGroup Trainium2 kernel (8 NeuronCores, SPMD).

Problem: for each (batch, query-box) pair, the reference multiplies the
in-box mask by arange(N) and takes the first 32 entries of a stable
ascending argsort. Because out-of-box points (and point 0) map to value 0,
that selects exactly the first 32 indices j with (j == 0 or point j outside
the box), in ascending order. local_group_mask is identically False (index 0
is always the first selected element and position 0 is forced False).

Kernel strategy (per core; core c handles batch c//2, query half c%2, so
128 (b,q) pairs per core, one per SBUF partition):
  - Box test on a K=128 point window -> z[q,j] (1 = selected candidate).
  - rank = prefix-sum(z); slot = z*rank-1 (negatives dropped);
    gpsimd local_scatter compacts the first 32 window indices per query.
  - Two PE transposes re-layout indices into ap_gather's wrapped format.
  - One gpsimd ap_gather per 1024-sample chunk pulls xyz+feature columns
    from a stacked [xyzT; features] SBUF tile; centers subtracted from the
    xyz rows; DMA straight out to the output slices.
The window is statistically enormous headroom (32nd candidate is at index
<= 38 for the reference distribution; P(window miss) ~ 1e-15 even for
adversarially-sized boxes). Each core also emits per-query candidate counts
within the window; any query with count < 32 (never in practice) is
recomputed exactly on the host.
"""
import sys
import numpy as np

for _p in ("/opt/trn_rl_repo", "/root/.axon_site/_ro/trn_rl_repo"):
    if _p not in sys.path:
        sys.path.insert(0, _p)

import ml_dtypes
import concourse.bass as bass
import concourse.mybir as mybir
import concourse.tile as tile
import concourse.bacc as bacc
from concourse.bass_utils import run_bass_kernel_spmd

f32 = mybir.dt.float32
bf16 = mybir.dt.bfloat16
i16 = mybir.dt.int16
Alu = mybir.AluOpType

B, N, C, NQ, NS = 4, 16384, 64, 256, 32
K = 128          # point scan window
NQ_CORE = 128    # (b,q) pairs per core
CH = 80          # gather tile partitions: 0..2 xyzT, 3..15 pad, 16..79 features
NSAMP_TOT = NQ_CORE * NS  # 4096
CHUNK = 1024
NCHUNK = NSAMP_TOT // CHUNK
N_CORES = 8


def _build(tc, outs, ins):
    nc = tc.nc
    nf_d, gx_d, counts_d = outs
    xk_d, xyzT_d, feat_d, qb_d, qbT3_d, jvals_d, ident_d = ins

    with tc.tile_pool(name="main", bufs=1) as pool, \
         tc.tile_pool(name="psum", bufs=1, space="PSUM") as psum:
        # ---- loads ----
        X = pool.tile([128, 3 * K], f32)
        nc.sync.dma_start(X[:, :], xk_d.partition_broadcast(128))

        qb = pool.tile([128, 6], f32)
        nc.sync.dma_start(qb[:, :], qb_d[:, :])

        G = pool.tile([CH, K], f32)
        nc.vector.memset(G[:, :], 0.0)
        nc.sync.dma_start(G[0:3, :], xyzT_d[:, :])
        nc.sync.dma_start(G[16:16 + C, :], feat_d[:, :])

        jv = pool.tile([128, K], bf16)
        nc.sync.dma_start(jv[:, :], jvals_d.partition_broadcast(128))

        ident = pool.tile([128, 128], bf16)
        nc.sync.dma_start(ident[:, :], ident_d[:, :])

        c_rep = pool.tile([3, NSAMP_TOT], f32)
        nc.sync.dma_start(c_rep[:, :], qbT3_d[:, :])

        # ---- per-query scalars ----
        h3 = pool.tile([128, 3], f32)
        nc.vector.tensor_scalar(h3[:, :], qb[:, 3:6], 0.5, None, Alu.mult)
        nh3 = pool.tile([128, 3], f32)
        nc.vector.tensor_scalar(nh3[:, :], qb[:, 3:6], -0.5, None, Alu.mult)

        def Xd(d):
            return X[:, :].rearrange("p (k d) -> p d k", d=3)[:, d, :]

        # ---- candidate mask: z = 1 iff outside box (z[:,0] forced 1) ----
        o1 = pool.tile([128, K], f32)
        nc.vector.tensor_scalar(o1[:, :], Xd(1), qb[:, 1:2], None, Alu.subtract)
        o2 = pool.tile([128, K], f32)
        nc.vector.tensor_scalar(o2[:, :], Xd(2), qb[:, 2:3], None, Alu.subtract)
        o0 = pool.tile([128, K], f32)
        nc.vector.tensor_scalar(o0[:, :], Xd(0), qb[:, 0:1], None, Alu.subtract)

        inb = pool.tile([128, K], f32)
        nc.vector.tensor_scalar(
            inb[:, :], Xd(0), qb[:, 0:1], h3[:, 0:1], Alu.subtract, Alu.is_le
        )
        for o, d in ((o0, 0), (o1, 1), (o2, 2)):
            if d != 0:
                nc.vector.scalar_tensor_tensor(
                    inb[:, :], o[:, :], h3[:, d:d + 1], inb[:, :],
                    Alu.is_le, Alu.logical_and,
                )
            nc.vector.scalar_tensor_tensor(
                inb[:, :], o[:, :], nh3[:, d:d + 1], inb[:, :],
                Alu.is_ge, Alu.logical_and,
            )
        z = pool.tile([128, K], f32)
        nc.vector.tensor_scalar(z[:, :], inb[:, :], 0.5, None, Alu.is_lt)
        nc.vector.memset(z[:, 0:1], 1.0)

        # ---- rank and slots ----
        rank = pool.tile([128, K], f32)
        nc.vector.tensor_tensor_scan(
            rank[:, :], z[:, :], z[:, :], 0.0, Alu.add, Alu.bypass
        )
        nc.sync.dma_start(counts_d[:, :], rank[:, K - 1:K])

        slotf = pool.tile([128, K], f32)
        nc.vector.tensor_tensor(slotf[:, :], z[:, :], rank[:, :], Alu.mult)
        slot16 = pool.tile([128, K], i16)
        nc.vector.tensor_scalar(slot16[:, :], slotf[:, :], 1.0, None, Alu.subtract)

        dst = pool.tile([128, K], bf16)
        nc.gpsimd.local_scatter(
            dst[:, :], jv[:, :], slot16[:, :],
            channels=128, num_elems=K, num_idxs=K,
        )

        # ---- wrapped index layout for ap_gather ----
        psA = psum.tile([16, 128], bf16)
        nc.tensor.transpose(psA[:, :], dst[:, 0:16], ident[:, :])
        psB = psum.tile([16, 128], bf16)
        nc.tensor.transpose(psB[:, :], dst[:, 16:32], ident[:, :])

        W16 = pool.tile([16, 2 * NQ_CORE], i16)
        nc.vector.tensor_copy(W16[:, 0:2 * NQ_CORE:2], psA[:, :])
        nc.vector.tensor_copy(W16[:, 1:2 * NQ_CORE:2], psB[:, :])

        W80 = pool.tile([CH, 2 * NQ_CORE], i16)
        for g in range(CH // 16):
            nc.sync.dma_start(W80[16 * g:16 * (g + 1), :], W16[:, :])

        # ---- gather + epilogue, chunked ----
        G_out = pool.tile([CH, NSAMP_TOT], f32)
        sub3 = pool.tile([3, NSAMP_TOT], f32)
        for ci in range(NCHUNK):
            lo, hi = ci * CHUNK, (ci + 1) * CHUNK
            qlo, qhi = ci * (CHUNK // NS), (ci + 1) * (CHUNK // NS)
            nc.gpsimd.ap_gather(
                G_out[:, lo:hi], G[:, :], W80[:, lo // 16:hi // 16],
                channels=CH, num_elems=K, d=1, num_idxs=CHUNK,
            )
            nc.vector.tensor_tensor(
                sub3[:, lo:hi], G_out[0:3, lo:hi], c_rep[:, lo:hi], Alu.subtract
            )
            nc.sync.dma_start(
                nf_d[3:3 + C, qlo:qhi, :], G_out[16:16 + C, lo:hi]
            )
            nc.sync.dma_start(nf_d[0:3, qlo:qhi, :], sub3[:, lo:hi])
            nc.sync.dma_start(gx_d[:, qlo:qhi, :], sub3[:, lo:hi])


_CACHE = {}


def _get_program():
    if "nc" in _CACHE:
        return _CACHE["nc"]
    nc = bacc.Bacc("TRN2", target_bir_lowering=False, debug=False)
    ins = [
        nc.dram_tensor("xk", [3 * K], f32, kind="ExternalInput").ap(),
        nc.dram_tensor("xyzT", [3, K], f32, kind="ExternalInput").ap(),
        nc.dram_tensor("feat", [C, K], f32, kind="ExternalInput").ap(),
        nc.dram_tensor("qb", [NQ_CORE, 6], f32, kind="ExternalInput").ap(),
        nc.dram_tensor("qbT3", [3, NQ_CORE * NS], f32, kind="ExternalInput").ap(),
        nc.dram_tensor("jvals", [K], bf16, kind="ExternalInput").ap(),
        nc.dram_tensor("ident", [128, 128], bf16, kind="ExternalInput").ap(),
    ]
    outs = [
        nc.dram_tensor("nf", [3 + C, NQ_CORE, NS], f32, kind="ExternalOutput").ap(),
        nc.dram_tensor("gx", [3, NQ_CORE, NS], f32, kind="ExternalOutput").ap(),
        nc.dram_tensor("counts", [NQ_CORE, 1], f32, kind="ExternalOutput").ap(),
    ]
    with tile.TileContext(nc) as tc:
        _build(tc, outs, ins)
    nc.compile()
    _CACHE["nc"] = nc
    return nc


def _in_maps(kx, kf, qbox):
    jvals = np.arange(K, dtype=ml_dtypes.bfloat16)
    ident = np.eye(128, dtype=ml_dtypes.bfloat16)
    per_b = {}
    for b in range(B):
        per_b[b] = dict(
            xk=np.ascontiguousarray(kx[b, :K, :].reshape(-1)),
            xyzT=np.ascontiguousarray(kx[b, :K, :].T),
            feat=np.ascontiguousarray(kf[b, :, :K]),
        )
    maps = []
    for core in range(N_CORES):
        b, half = core // 2, core % 2
        qs = np.ascontiguousarray(qbox[b, half * NQ_CORE:(half + 1) * NQ_CORE, :])
        m = dict(per_b[b])
        m.update(
            qb=qs,
            qbT3=np.ascontiguousarray(
                np.repeat(qs[:, :3].T[:, :, None], NS, axis=2).reshape(3, -1)),
            jvals=jvals,
            ident=ident,
        )
        maps.append(m)
    return maps


def _host_fix_query(kx_b, kf_b, qbox_bq, nf_bq, gx_bq):
    """Exact recompute of one (b, q) pair on the host (window fallback)."""
    center, size = qbox_bq[:3], qbox_bq[3:]
    off = np.abs(kx_b - center[None, :])
    inb = (off <= size[None, :] * 0.5).all(-1)
    z = ~inb
    z[0] = True
    zi = np.flatnonzero(z)[:NS]
    gxq = kx_b[zi, :].T - center[:, None]
    gx_bq[:] = gxq
    nf_bq[0:3] = gxq
    nf_bq[3:] = kf_b[:, zi]


def kernel(key_xyz, key_features, query_box, _want_timing=False):
    kx = np.ascontiguousarray(np.asarray(key_xyz, dtype=np.float32))
    kf = np.ascontiguousarray(np.asarray(key_features, dtype=np.float32))
    qbox = np.ascontiguousarray(np.asarray(query_box, dtype=np.float32))
    assert kx.shape == (B, N, 3) and kf.shape == (B, C, N) and qbox.shape == (B, NQ, 6)

    nc = _get_program()
    res = run_bass_kernel_spmd(nc, _in_maps(kx, kf, qbox), list(range(N_CORES)))

    grouped_xyz = np.empty((B, 3, NQ, NS), np.float32)
    new_features = np.empty((B, 3 + C, NQ, NS), np.float32)
    mask = np.zeros((B, NQ, NS), dtype=bool)
    for core in range(N_CORES):
        b, half = core // 2, core % 2
        sl = slice(half * NQ_CORE, (half + 1) * NQ_CORE)
        r = res.results[core]
        new_features[b, :, sl, :] = r["nf"]
        grouped_xyz[b, :, sl, :] = r["gx"]
        counts = r["counts"][:, 0]
        if (counts < NS).any():
            for q in np.flatnonzero(counts < NS):
                gq = half * NQ_CORE + int(q)
                _host_fix_query(
                    kx[b], kf[b], qbox[b, gq],
                    new_features[b, :, gq, :], grouped_xyz[b, :, gq, :],
                )
    out = (grouped_xyz, new_features, mask)
    if _want_timing:
        return out, res
    return out


# revision 5
# speedup vs baseline: 1.7388x; 1.7388x over previous
"""BoxQueryAndGroup Trainium2 kernel (8 NeuronCores, SPMD).

Problem: for each (batch, query-box) pair, the reference multiplies the
in-box mask by arange(N) and takes the first 32 entries of a stable
ascending argsort. Because out-of-box points (and point 0) map to value 0,
that selects exactly the first 32 indices j with (j == 0 or point j outside
the box), in ascending order. local_group_mask is identically False (index 0
is always the first selected element and position 0 is forced False).

Kernel strategy (per core; core c handles batch c//2, query half c%2, so
128 (b,q) pairs per core, one per SBUF partition):
  - Box test on a K=128 point window -> z[q,j] (1 = selected candidate).
  - rank = prefix-sum(z); slot = z*rank-1 (negatives dropped);
    gpsimd local_scatter compacts the first 32 window indices per query.
  - Two PE transposes re-layout indices into the SWDGE wrapped format.
  - dma_gather pulls 512B combined [xyz | features] rows (host-packed for
    the K-window) from HBM per 1024-sample chunk; PE transposes flip each
    [sample, channel] block into channel-major; centers are subtracted from
    the xyz rows in PSUM; results DMA straight out to the output slices.
The window is statistically enormous headroom (32nd candidate is at index
<= 38 for the reference distribution; P(window miss) ~ 1e-15 even for
adversarially-sized boxes). Each core also emits per-query candidate counts
within the window; any query with count < 32 (never in practice) is
recomputed exactly on the host.
"""
import sys
import numpy as np

for _p in ("/opt/trn_rl_repo", "/root/.axon_site/_ro/trn_rl_repo"):
    if _p not in sys.path:
        sys.path.insert(0, _p)

import ml_dtypes
import concourse.bass as bass
import concourse.mybir as mybir
import concourse.tile as tile
import concourse.bacc as bacc
from concourse.bass_utils import run_bass_kernel_spmd

f32 = mybir.dt.float32
bf16 = mybir.dt.bfloat16
i16 = mybir.dt.int16
Alu = mybir.AluOpType

B, N, C, NQ, NS = 4, 16384, 64, 256, 32
K = 128          # point scan window
NQ_CORE = 128    # (b,q) pairs per core
CH = 80          # gather tile partitions: 0..2 xyzT, 3..15 pad, 16..79 features
NSAMP_TOT = NQ_CORE * NS  # 4096
CHUNK = 1024
NCHUNK = NSAMP_TOT // CHUNK
N_CORES = 8


def _build(tc, outs, ins):
    nc = tc.nc
    nf_d, gx_d, counts_d = outs
    xk_d, rows_d, qb_d, qbT3_d, jvals_d, ident_d, identF_d = ins

    with tc.tile_pool(name="main", bufs=1) as pool, \
         tc.tile_pool(name="psum", bufs=1, space="PSUM") as psum:
        # ---- loads ----
        X = pool.tile([128, 3 * K], f32)
        nc.sync.dma_start(X[:, :], xk_d.partition_broadcast(128))

        qb = pool.tile([128, 6], f32)
        nc.sync.dma_start(qb[:, :], qb_d[:, :])

        jv = pool.tile([128, K], bf16)
        nc.sync.dma_start(jv[:, :], jvals_d.partition_broadcast(128))

        ident = pool.tile([128, 128], bf16)
        nc.sync.dma_start(ident[:, :], ident_d[:, :])

        identF = pool.tile([128, 128], f32)
        nc.sync.dma_start(identF[:, :], identF_d[:, :])

        c_rep = pool.tile([3, NSAMP_TOT], f32)
        nc.sync.dma_start(c_rep[:, :], qbT3_d[:, :])

        # ---- per-query scalars ----
        h3 = pool.tile([128, 3], f32)
        nc.vector.tensor_scalar(h3[:, :], qb[:, 3:6], 0.5, None, Alu.mult)
        nh3 = pool.tile([128, 3], f32)
        nc.vector.tensor_scalar(nh3[:, :], qb[:, 3:6], -0.5, None, Alu.mult)

        def Xd(d):
            return X[:, :].rearrange("p (k d) -> p d k", d=3)[:, d, :]

        # ---- candidate mask: z = 1 iff outside box (z[:,0] forced 1) ----
        o1 = pool.tile([128, K], f32)
        nc.vector.tensor_scalar(o1[:, :], Xd(1), qb[:, 1:2], None, Alu.subtract)
        o2 = pool.tile([128, K], f32)
        nc.vector.tensor_scalar(o2[:, :], Xd(2), qb[:, 2:3], None, Alu.subtract)
        o0 = pool.tile([128, K], f32)
        nc.vector.tensor_scalar(o0[:, :], Xd(0), qb[:, 0:1], None, Alu.subtract)

        inb = pool.tile([128, K], f32)
        nc.vector.tensor_scalar(
            inb[:, :], Xd(0), qb[:, 0:1], h3[:, 0:1], Alu.subtract, Alu.is_le
        )
        for o, d in ((o0, 0), (o1, 1), (o2, 2)):
            if d != 0:
                nc.vector.scalar_tensor_tensor(
                    inb[:, :], o[:, :], h3[:, d:d + 1], inb[:, :],
                    Alu.is_le, Alu.logical_and,
                )
            nc.vector.scalar_tensor_tensor(
                inb[:, :], o[:, :], nh3[:, d:d + 1], inb[:, :],
                Alu.is_ge, Alu.logical_and,
            )
        z = pool.tile([128, K], f32)
        nc.vector.tensor_scalar(z[:, :], inb[:, :], 0.5, None, Alu.is_lt)
        nc.vector.memset(z[:, 0:1], 1.0)

        # ---- rank and slots ----
        rank = pool.tile([128, K], f32)
        nc.vector.tensor_tensor_scan(
            rank[:, :], z[:, :], z[:, :], 0.0, Alu.add, Alu.bypass
        )
        nc.sync.dma_start(counts_d[:, :], rank[:, K - 1:K])

        slotf = pool.tile([128, K], f32)
        nc.vector.tensor_tensor(slotf[:, :], z[:, :], rank[:, :], Alu.mult)
        slot16 = pool.tile([128, K], i16)
        nc.vector.tensor_scalar(slot16[:, :], slotf[:, :], 1.0, None, Alu.subtract)

        dst = pool.tile([128, K], bf16)
        nc.gpsimd.local_scatter(
            dst[:, :], jv[:, :], slot16[:, :],
            channels=128, num_elems=K, num_idxs=K,
        )

        # ---- wrapped index layout for ap_gather ----
        psA = psum.tile([16, 128], bf16)
        nc.tensor.transpose(psA[:, :], dst[:, 0:16], ident[:, :])
        psB = psum.tile([16, 128], bf16)
        nc.tensor.transpose(psB[:, :], dst[:, 16:32], ident[:, :])

        W16 = pool.tile([16, 2 * NQ_CORE], i16)
        nc.vector.tensor_copy(W16[:, 0:2 * NQ_CORE:2], psA[:, :])
        nc.vector.tensor_copy(W16[:, 1:2 * NQ_CORE:2], psB[:, :])

        W128 = pool.tile([128, 2 * NQ_CORE], i16)
        for g in range(8):
            nc.sync.dma_start(W128[16 * g:16 * (g + 1), :], W16[:, :])

        # ---- gather + transpose + epilogue, chunked ----
        with tc.tile_pool(name="gg", bufs=2) as ggpool, \
             tc.tile_pool(name="pst", bufs=2, space="PSUM") as pstpool, \
             tc.tile_pool(name="sub", bufs=2) as subpool:
            for ci in range(NCHUNK):
                lo, hi = ci * CHUNK, (ci + 1) * CHUNK
                qlo, qhi = ci * (CHUNK // NS), (ci + 1) * (CHUNK // NS)
                gg = ggpool.tile([128, CHUNK // 128, 128], f32)
                nc.gpsimd.dma_gather(
                    gg[:, :, :], rows_d[:, :], W128[:, lo // 16:hi // 16],
                    num_idxs=CHUNK, num_idxs_reg=CHUNK, elem_size=128,
                )
                psT = pstpool.tile([128, CHUNK], f32)
                for j in range(CHUNK // 128):
                    nc.tensor.transpose(
                        psT[:, 128 * j:128 * (j + 1)], gg[:, j, :], identF[:, :]
                    )
                sub3 = subpool.tile([3, CHUNK], f32)
                nc.vector.tensor_tensor(
                    sub3[:, :], psT[0:3, :], c_rep[:, lo:hi], Alu.subtract
                )
                fsb = subpool.tile([16 + C, CHUNK], f32)
                nc.scalar.copy(fsb[:, :], psT[0:16 + C, :])
                nc.sync.dma_start(nf_d[3:3 + C, qlo:qhi, :], fsb[16:16 + C, :])
                nc.sync.dma_start(nf_d[0:3, qlo:qhi, :], sub3[:, :])
                nc.sync.dma_start(gx_d[:, qlo:qhi, :], sub3[:, :])


_CACHE = {}


def _get_program():
    if "nc" in _CACHE:
        return _CACHE["nc"]
    nc = bacc.Bacc("TRN2", target_bir_lowering=False, debug=False)
    ins = [
        nc.dram_tensor("xk", [3 * K], f32, kind="ExternalInput").ap(),
        nc.dram_tensor("rows", [K, 128], f32, kind="ExternalInput").ap(),
        nc.dram_tensor("qb", [NQ_CORE, 6], f32, kind="ExternalInput").ap(),
        nc.dram_tensor("qbT3", [3, NQ_CORE * NS], f32, kind="ExternalInput").ap(),
        nc.dram_tensor("jvals", [K], bf16, kind="ExternalInput").ap(),
        nc.dram_tensor("ident", [128, 128], bf16, kind="ExternalInput").ap(),
        nc.dram_tensor("identF", [128, 128], f32, kind="ExternalInput").ap(),
    ]
    outs = [
        nc.dram_tensor("nf", [3 + C, NQ_CORE, NS], f32, kind="ExternalOutput").ap(),
        nc.dram_tensor("gx", [3, NQ_CORE, NS], f32, kind="ExternalOutput").ap(),
        nc.dram_tensor("counts", [NQ_CORE, 1], f32, kind="ExternalOutput").ap(),
    ]
    with tile.TileContext(nc) as tc:
        _build(tc, outs, ins)
    nc.compile()
    _CACHE["nc"] = nc
    return nc


def _in_maps(kx, kf, qbox):
    jvals = np.arange(K, dtype=ml_dtypes.bfloat16)
    ident = np.eye(128, dtype=ml_dtypes.bfloat16)
    identF = np.eye(128, dtype=np.float32)
    per_b = {}
    for b in range(B):
        rows = np.zeros((K, 128), np.float32)
        rows[:, 0:3] = kx[b, :K, :]
        rows[:, 16:16 + C] = kf[b, :, :K].T
        per_b[b] = dict(
            xk=np.ascontiguousarray(kx[b, :K, :].reshape(-1)),
            rows=rows,
        )
    maps = []
    for core in range(N_CORES):
        b, half = core // 2, core % 2
        qs = np.ascontiguousarray(qbox[b, half * NQ_CORE:(half + 1) * NQ_CORE, :])
        m = dict(per_b[b])
        m.update(
            qb=qs,
            qbT3=np.ascontiguousarray(
                np.repeat(qs[:, :3].T[:, :, None], NS, axis=2).reshape(3, -1)),
            jvals=jvals,
            ident=ident,
            identF=identF,
        )
        maps.append(m)
    return maps


def _host_fix_query(kx_b, kf_b, qbox_bq, nf_bq, gx_bq):
    """Exact recompute of one (b, q) pair on the host (window fallback)."""
    center, size = qbox_bq[:3], qbox_bq[3:]
    off = np.abs(kx_b - center[None, :])
    inb = (off <= size[None, :] * 0.5).all(-1)
    z = ~inb
    z[0] = True
    zi = np.flatnonzero(z)[:NS]
    gxq = kx_b[zi, :].T - center[:, None]
    gx_bq[:] = gxq
    nf_bq[0:3] = gxq
    nf_bq[3:] = kf_b[:, zi]


def kernel(key_xyz, key_features, query_box, _want_timing=False):
    kx = np.ascontiguousarray(np.asarray(key_xyz, dtype=np.float32))
    kf = np.ascontiguousarray(np.asarray(key_features, dtype=np.float32))
    qbox = np.ascontiguousarray(np.asarray(query_box, dtype=np.float32))
    assert kx.shape == (B, N, 3) and kf.shape == (B, C, N) and qbox.shape == (B, NQ, 6)

    nc = _get_program()
    res = run_bass_kernel_spmd(nc, _in_maps(kx, kf, qbox), list(range(N_CORES)))

    grouped_xyz = np.empty((B, 3, NQ, NS), np.float32)
    new_features = np.empty((B, 3 + C, NQ, NS), np.float32)
    mask = np.zeros((B, NQ, NS), dtype=bool)
    for core in range(N_CORES):
        b, half = core // 2, core % 2
        sl = slice(half * NQ_CORE, (half + 1) * NQ_CORE)
        r = res.results[core]
        new_features[b, :, sl, :] = r["nf"]
        grouped_xyz[b, :, sl, :] = r["gx"]
        counts = r["counts"][:, 0]
        if (counts < NS).any():
            for q in np.flatnonzero(counts < NS):
                gq = half * NQ_CORE + int(q)
                _host_fix_query(
                    kx[b], kf[b], qbox[b, gq],
                    new_features[b, :, gq, :], grouped_xyz[b, :, gq, :],
                )
    out = (grouped_xyz, new_features, mask)
    if _want_timing:
        return out, res
    return out


# revision 8
# speedup vs baseline: 2.3227x; 1.3359x over previous
"""BoxQueryAndGroup Trainium2 kernel (8 NeuronCores, SPMD).

Problem: for each (batch, query-box) pair, the reference multiplies the
in-box mask by arange(N) and takes the first 32 entries of a stable
ascending argsort. Because out-of-box points (and point 0) map to value 0,
that selects exactly the first 32 indices j with (j == 0 or point j outside
the box), in ascending order. local_group_mask is identically False (index 0
is always the first selected element and position 0 is forced False).

Kernel strategy (per core; core c handles batch c//2, query half c%2, so
128 (b,q) pairs per core, one per SBUF partition):
  - Box test on a K=128 point window -> z[q,j] (1 = selected candidate).
  - v = z * (K - j); 4 rounds of DVE max8/max_index/match_replace extract
    the 32 largest v per query = the first 32 candidate indices, ascending.
  - Indices are flattened to one row by DMA, broadcast across partitions by
    a K=1 PE matmul, and turned into an exact one-hot via is_equal against
    the partition id.
  - The gather itself is an fp32 PE matmul: rows.T @ onehot, where rows is
    a host-packed [K, 128] array with xyz in cols 0..2 and features in cols
    16..79. fp32 weights go through the PE's exact hi/lo bit-split, so the
    selected values are reproduced bit-exactly (verified on HW).
  - ScalarE evacuates PSUM; GpSimd subtracts box centers from the xyz rows;
    outputs DMA out in 1024-sample batches over multiple DGE queues.
The window has enormous statistical headroom (32nd candidate at index <= 38
for the reference distribution). Each core also emits per-query candidate
counts within the window; any query with count < 32 (never in practice) is
recomputed exactly on the host.
"""
import sys
import numpy as np

for _p in ("/opt/trn_rl_repo", "/root/.axon_site/_ro/trn_rl_repo"):
    if _p not in sys.path:
        sys.path.insert(0, _p)

import concourse.bass as bass
import concourse.mybir as mybir
import concourse.tile as tile
import concourse.bacc as bacc
from concourse.bass_utils import run_bass_kernel_spmd

f32 = mybir.dt.float32
u32 = mybir.dt.uint32
Alu = mybir.AluOpType

B, N, C, NQ, NS = 4, 16384, 64, 256, 32
K = 128          # point scan window
NQ_CORE = 128    # (b,q) pairs per core
NSAMP_TOT = NQ_CORE * NS  # 4096
MMCHUNK = 512             # samples per matmul chunk (one PSUM bank)
OUTBATCH = 1024           # samples per output DMA batch
N_CORES = 8


def _build(tc, outs, ins):
    nc = tc.nc
    nf_d, gx_d, counts_d = outs
    xk_d, rows_d, qb_d, qbT3_d, vdesc_d, iotap_d = ins

    with tc.tile_pool(name="main", bufs=1) as pool, \
         tc.tile_pool(name="psum", bufs=1, space="PSUM") as psum:
        # ---- loads (spread across DGE queues) ----
        X = pool.tile([128, 3 * K], f32)
        nc.sync.dma_start(X[:, :], xk_d.partition_broadcast(128))

        qb = pool.tile([128, 6], f32)
        nc.sync.dma_start(qb[:, :], qb_d[:, :])

        rows = pool.tile([K, 128], f32)
        nc.scalar.dma_start(rows[:, :], rows_d[:, :])

        c_rep = pool.tile([3, NSAMP_TOT], f32)
        nc.scalar.dma_start(c_rep[:, :], qbT3_d[:, :])

        vdesc = pool.tile([128, K], f32)
        nc.scalar.dma_start(vdesc[:, :], vdesc_d.partition_broadcast(128))

        iotap = pool.tile([128, 1], f32)
        nc.scalar.dma_start(iotap[:, :], iotap_d[:, :])

        ones = pool.tile([1, 128], f32)
        nc.vector.memset(ones[:, :], 1.0)

        # ---- per-query scalars ----
        h3 = pool.tile([128, 3], f32)
        nc.vector.tensor_scalar(h3[:, :], qb[:, 3:6], 0.5, None, Alu.mult)
        nh3 = pool.tile([128, 3], f32)
        nc.vector.tensor_scalar(nh3[:, :], qb[:, 3:6], -0.5, None, Alu.mult)

        def Xd(d):
            return X[:, :].rearrange("p (k d) -> p d k", d=3)[:, d, :]

        # ---- candidate mask: z = 1 iff outside box (z[:,0] forced 1) ----
        o1 = pool.tile([128, K], f32)
        nc.vector.tensor_scalar(o1[:, :], Xd(1), qb[:, 1:2], None, Alu.subtract)
        o2 = pool.tile([128, K], f32)
        nc.vector.tensor_scalar(o2[:, :], Xd(2), qb[:, 2:3], None, Alu.subtract)
        o0 = pool.tile([128, K], f32)
        nc.vector.tensor_scalar(o0[:, :], Xd(0), qb[:, 0:1], None, Alu.subtract)

        inb = pool.tile([128, K], f32)
        nc.vector.tensor_scalar(
            inb[:, :], Xd(0), qb[:, 0:1], h3[:, 0:1], Alu.subtract, Alu.is_le
        )
        for o, d in ((o0, 0), (o1, 1), (o2, 2)):
            if d != 0:
                nc.vector.scalar_tensor_tensor(
                    inb[:, :], o[:, :], h3[:, d:d + 1], inb[:, :],
                    Alu.is_le, Alu.logical_and,
                )
            nc.vector.scalar_tensor_tensor(
                inb[:, :], o[:, :], nh3[:, d:d + 1], inb[:, :],
                Alu.is_ge, Alu.logical_and,
            )
        z = pool.tile([128, K], f32)
        nc.vector.tensor_scalar(z[:, :], inb[:, :], 0.5, None, Alu.is_lt)
        nc.vector.memset(z[:, 0:1], 1.0)

        counts = pool.tile([128, 1], f32)
        nc.vector.reduce_sum(counts[:, :], z[:, :], axis=mybir.AxisListType.X)
        nc.sync.dma_start(counts_d[:, :], counts[:, :])

        # ---- first-32 selection: v = z*(K-j); 4x max8 rounds ----
        v = pool.tile([128, K], f32)
        nc.vector.tensor_tensor(v[:, :], z[:, :], vdesc[:, :], Alu.mult)

        offs32 = pool.tile([128, NS], u32)
        for r in range(4):
            mx = pool.tile([128, 8], f32, tag="mx")
            nc.vector.max(mx[:, :], v[:, :])
            nc.vector.max_index(offs32[:, 8 * r:8 * r + 8], mx[:, :], v[:, :])
            if r < 3:
                nc.vector.match_replace(v[:, :], mx[:, :], v[:, :], 0.0)

        idxf = pool.tile([128, NS], f32)
        nc.vector.tensor_copy(idxf[:, :], offs32[:, :])
        idxrow = pool.tile([1, NSAMP_TOT], f32)
        nc.sync.dma_start(idxrow[:, :], idxf[:, :])

        # ---- one-hot matmul gather, chunked ----
        with tc.tile_pool(name="oh", bufs=3) as ohpool, \
             tc.tile_pool(name="ps", bufs=4, space="PSUM") as pst, \
             tc.tile_pool(name="ob", bufs=2) as obpool:
            nbat = NSAMP_TOT // OUTBATCH
            per = OUTBATCH // MMCHUNK
            for bi in range(nbat):
                fsb = obpool.tile([16 + C, OUTBATCH], f32, tag="fsb")
                sub3 = obpool.tile([3, OUTBATCH], f32, tag="sub3")
                for cj in range(per):
                    lo = bi * OUTBATCH + cj * MMCHUNK
                    psI = pst.tile([128, MMCHUNK], f32, tag="psI")
                    nc.tensor.matmul(
                        psI[:, :], ones[:, :], idxrow[:, lo:lo + MMCHUNK]
                    )
                    oh = ohpool.tile([128, MMCHUNK], f32)
                    nc.vector.tensor_scalar(
                        oh[:, :], psI[:, :], iotap[:, 0:1], None, Alu.is_equal
                    )
                    psG = pst.tile([128, MMCHUNK], f32, tag="psG")
                    nc.tensor.matmul(psG[:, :], rows[:, :], oh[:, :])
                    sl = slice(cj * MMCHUNK, (cj + 1) * MMCHUNK)
                    nc.scalar.copy(fsb[:, sl], psG[0:16 + C, :])
                    nc.gpsimd.tensor_tensor(
                        sub3[:, sl], fsb[0:3, sl],
                        c_rep[:, lo:lo + MMCHUNK], Alu.subtract,
                    )
                qlo, qhi = bi * (OUTBATCH // NS), (bi + 1) * (OUTBATCH // NS)
                nc.sync.dma_start(nf_d[3:3 + C, qlo:qhi, :], fsb[16:16 + C, :])
                nc.scalar.dma_start(nf_d[0:3, qlo:qhi, :], sub3[:, :])
                nc.sync.dma_start(gx_d[:, qlo:qhi, :], sub3[:, :])


_CACHE = {}


def _get_program():
    if "nc" in _CACHE:
        return _CACHE["nc"]
    nc = bacc.Bacc("TRN2", target_bir_lowering=False, debug=False)
    ins = [
        nc.dram_tensor("xk", [3 * K], f32, kind="ExternalInput").ap(),
        nc.dram_tensor("rows", [K, 128], f32, kind="ExternalInput").ap(),
        nc.dram_tensor("qb", [NQ_CORE, 6], f32, kind="ExternalInput").ap(),
        nc.dram_tensor("qbT3", [3, NQ_CORE * NS], f32, kind="ExternalInput").ap(),
        nc.dram_tensor("vdesc", [K], f32, kind="ExternalInput").ap(),
        nc.dram_tensor("iotap", [128, 1], f32, kind="ExternalInput").ap(),
    ]
    outs = [
        nc.dram_tensor("nf", [3 + C, NQ_CORE, NS], f32, kind="ExternalOutput").ap(),
        nc.dram_tensor("gx", [3, NQ_CORE, NS], f32, kind="ExternalOutput").ap(),
        nc.dram_tensor("counts", [NQ_CORE, 1], f32, kind="ExternalOutput").ap(),
    ]
    with tile.TileContext(nc) as tc:
        _build(tc, outs, ins)
    nc.compile()
    _CACHE["nc"] = nc
    return nc


def _in_maps(kx, kf, qbox):
    vdesc = (K - np.arange(K)).astype(np.float32)
    iotap = np.arange(128, dtype=np.float32).reshape(128, 1)
    per_b = {}
    for b in range(B):
        rows = np.zeros((K, 128), np.float32)
        rows[:, 0:3] = kx[b, :K, :]
        rows[:, 16:16 + C] = kf[b, :, :K].T
        per_b[b] = dict(
            xk=np.ascontiguousarray(kx[b, :K, :].reshape(-1)),
            rows=rows,
        )
    maps = []
    for core in range(N_CORES):
        b, half = core // 2, core % 2
        qs = np.ascontiguousarray(qbox[b, half * NQ_CORE:(half + 1) * NQ_CORE, :])
        m = dict(per_b[b])
        m.update(
            qb=qs,
            qbT3=np.ascontiguousarray(
                np.repeat(qs[:, :3].T[:, :, None], NS, axis=2).reshape(3, -1)),
            vdesc=vdesc,
            iotap=iotap,
        )
        maps.append(m)
    return maps


def _host_fix_query(kx_b, kf_b, qbox_bq, nf_bq, gx_bq):
    """Exact recompute of one (b, q) pair on the host (window fallback)."""
    center, size = qbox_bq[:3], qbox_bq[3:]
    off = np.abs(kx_b - center[None, :])
    inb = (off <= size[None, :] * 0.5).all(-1)
    z = ~inb
    z[0] = True
    zi = np.flatnonzero(z)[:NS]
    gxq = kx_b[zi, :].T - center[:, None]
    gx_bq[:] = gxq
    nf_bq[0:3] = gxq
    nf_bq[3:] = kf_b[:, zi]


def kernel(key_xyz, key_features, query_box, _want_timing=False):
    kx = np.ascontiguousarray(np.asarray(key_xyz, dtype=np.float32))
    kf = np.ascontiguousarray(np.asarray(key_features, dtype=np.float32))
    qbox = np.ascontiguousarray(np.asarray(query_box, dtype=np.float32))
    assert kx.shape == (B, N, 3) and kf.shape == (B, C, N) and qbox.shape == (B, NQ, 6)

    nc = _get_program()
    res = run_bass_kernel_spmd(nc, _in_maps(kx, kf, qbox), list(range(N_CORES)))

    grouped_xyz = np.empty((B, 3, NQ, NS), np.float32)
    new_features = np.empty((B, 3 + C, NQ, NS), np.float32)
    mask = np.zeros((B, NQ, NS), dtype=bool)
    for core in range(N_CORES):
        b, half = core // 2, core % 2
        sl = slice(half * NQ_CORE, (half + 1) * NQ_CORE)
        r = res.results[core]
        new_features[b, :, sl, :] = r["nf"]
        grouped_xyz[b, :, sl, :] = r["gx"]
        counts = r["counts"][:, 0]
        if (counts < NS).any():
            for q in np.flatnonzero(counts < NS):
                gq = half * NQ_CORE + int(q)
                _host_fix_query(
                    kx[b], kf[b], qbox[b, gq],
                    new_features[b, :, gq, :], grouped_xyz[b, :, gq, :],
                )
    out = (grouped_xyz, new_features, mask)
    if _want_timing:
        return out, res
    return out
